# revision 26
# baseline (speedup 1.0000x reference)
"""Trainium2 Bass kernel for PiecewiseLinearUnitV2 (elementwise piecewise-linear unit).

Contract: kernel(**inputs) takes the FULL (unsharded) numpy inputs and returns
the FULL output. Internally the input batch is data-parallel sharded across 8
NeuronCores; the ~25-float parameter tensors are folded into compile-time
immediates on the host.

Math: the reference computes, per element x,
    y = b1*l1 + b2*l2 + b3*l3
with uniform bins between Bounds[0]=Bl and Bounds[1]=Br. That is a piecewise
linear function of x: continuous at Bl and at all interior knots, with a jump
J = nheight[I+1] - nheight[I] at Br. So it decomposes exactly as
    y = Kl*x + (nh0 - Kl*Bl)
        + sum_k d_k * relu(x - c_k)        (slope changes at Bl + k*IL)
        + (Kr - s_{I-1}) * relu(x - Br)
        + J * (x >= Br)
Terms with negligible |d_k| are dropped (for linspace nheight all interior
slope-changes vanish, leaving a 3-piece function). The relus run on ScalarE
(ACT) with the coefficient folded into the activation scale/bias; the jump
mask is one fused tensor_scalar (is_ge, mult) on VectorE at 2x mode; sums are
in-place tensor_tensor adds on VectorE.
"""

import numpy as np

P = 128
N_CORES = 8
MAX_N = 20

# Set by test harness to request an NTFF profile; results land in LAST_RESULTS.
TRACE = False
LAST_RESULTS = None

_PROGRAM_CACHE = {}


def _plan_params(N, Bounds, BoundSlope, nheight):
    """Mirror the reference's float32 arithmetic to derive the relu-sum
    coefficients. Returns (terms, base, jump) with plain-float entries:
      terms: [(d, c)]  ->  d * relu(x - c)
      base:  (a, b)    ->  a*x + b        (None if exactly zero)
      jump:  (Br, J)   ->  J * (x >= Br)  (None if J == 0)
    """
    f32 = np.float32
    intervals = f32(np.floor(np.clip(f32(N), f32(3.0), f32(MAX_N))))
    I = int(intervals)
    Bl, Br = f32(Bounds[0]), f32(Bounds[1])
    Kl, Kr = f32(BoundSlope[0]), f32(BoundSlope[1])
    nh = np.asarray(nheight, dtype=np.float32)
    IL = f32((Br - Bl) / intervals)

    s = [f32((nh[k + 1] - nh[k]) / IL) for k in range(I)]
    cs = [f32(f32(k) * IL + Bl) for k in range(I)] + [Br]
    ds = [f32(s[0] - Kl)] + [f32(s[k] - s[k - 1]) for k in range(1, I)]
    ds.append(f32(Kr - s[I - 1]))
    # jnp clamps out-of-bounds gathers, so nheight[I+1] at I==MAX_N reads nh[MAX_N]
    J = f32(nh[min(I + 1, MAX_N)] - nh[I])

    dmax = max([abs(float(d)) for d in ds] + [1e-30])
    terms = [
        (float(d), float(c))
        for d, c in zip(ds, cs)
        if abs(float(d)) > 1e-6 * max(dmax, 1.0)
    ]
    base_a = float(Kl)
    base_b = float(f32(nh[0] - f32(Kl * Bl)))
    base = None if (base_a == 0.0 and base_b == 0.0) else (base_a, base_b)
    jump = None if float(J) == 0.0 else (float(Br), float(J))
    return terms, base, jump


def _pick_tile_free_dim(FT, n_slots, budget_bytes=int(22.5 * 1024 * 1024)):
    """Largest even divisor of FT such that n_slots tiles of [128, F] f32 fit
    in the SBUF budget."""
    fmax = budget_bytes // (P * 4 * n_slots)
    best = 0
    f = 2
    while f <= FT:
        if FT % f == 0 and f <= fmax and f <= 8192:
            best = max(best, f)
        f += 2
    assert best > 0, f"no usable tile size for FT={FT}, slots={n_slots}"
    return best


# Tile sizing (bench.py overrides these for experiments). Measured on HW:
# F=6272 with 3 input bufs / 2 relu bufs runs at the HBM roofline (~71us/core);
# smaller tiles pay per-instruction gaps on DVE/ACT.
F_OVERRIDE = None
BUFS = 2
BUFS_X = 3
BUFS_R = None


def _build_program(terms, base, jump, FT, repeat=1):
    from contextlib import ExitStack

    import concourse.bass as bass
    import concourse.tile as tile
    from concourse import bacc
    import concourse.mybir as mybir

    Alu = mybir.AluOpType
    Act = mybir.ActivationFunctionType
    f32 = mybir.dt.float32
    f32np = np.float32

    bufs_x = BUFS_X or BUFS
    bufs_r = BUFS_R or BUFS
    # SBUF slot count: x + per-relu tiles + 2 for the misc pool
    n_slots = bufs_x + bufs_r * max(len(terms), 1) + 2 * (
        (jump is not None) + (base is not None)
    )
    F = F_OVERRIDE or _pick_tile_free_dim(FT, n_slots)
    n_tiles = FT // F

    nc = bacc.Bacc("TRN2", target_bir_lowering=False, debug=False, num_devices=N_CORES)
    x_d = nc.dram_tensor("x", [P, FT], f32, kind="ExternalInput").ap()
    y_d = nc.dram_tensor("y", [P, FT], f32, kind="ExternalOutput").ap()

    with tile.TileContext(nc) as tc, ExitStack() as ctx:
        xpool = ctx.enter_context(tc.tile_pool(name="xpool", bufs=bufs_x))
        rpool = ctx.enter_context(tc.tile_pool(name="rpool", bufs=bufs_r))
        mpool = ctx.enter_context(tc.tile_pool(name="mpool", bufs=2))
        cpool = ctx.enter_context(tc.tile_pool(name="cpool", bufs=1))

        # Decide the ACT scale for each relu term. If there is a jump, the
        # LAST term's relu is emitted pre-divided by J so the jump mask can be
        # fused in for free:
        #   t = (x >= Br) +/- relu(|d/J|*x + bias)      (one stt, is_ge+add)
        #   y = (t * J) + next                          (one stt, mult+add)
        # All other terms carry |d| inside the ACT scale and are combined with
        # add/sub tensor_tensor ops.
        fold_jump = (
            jump is not None
            and len(terms) > 0
            and 1e-4 <= abs(terms[-1][0] / jump[1]) <= 1e4
        )
        act_scales = []  # (scale, bias, sign_for_combine)
        for j, (d, c) in enumerate(terms):
            if fold_jump and j == len(terms) - 1:
                ratio = f32np(d) / f32np(jump[1])
                sc = abs(ratio)
                sign = 1 if ratio > 0 else -1
            else:
                sc = abs(f32np(d))
                sign = 1 if d > 0 else -1
            bi = -f32np(f32np(sc) * f32np(c))
            act_scales.append((float(sc), float(bi), sign))

        # per-term [P,1] bias tiles for the ACT relus (bias must be an AP)
        bias_tiles = []
        for j, (sc, bi, sign) in enumerate(act_scales):
            bias_t = cpool.tile([P, 1], f32, name=f"bias{j}", tag=f"bias{j}")
            nc.vector.memset(bias_t[:], float(bi))
            bias_tiles.append(bias_t)

        for i in [t for _ in range(repeat) for t in range(n_tiles)]:
            xt = xpool.tile([P, F], f32, name="xt", tag="xt")
            nc.sync.dma_start(xt[:], x_d[:, bass.ts(i, F)])

            relu_tiles = []
            for j, (sc, bi, sign) in enumerate(act_scales):
                rt = rpool.tile([P, F], f32, name=f"rt{j}", tag=f"rt{j}", bufs=bufs_r)
                nc.scalar.activation(
                    rt[:], xt[:], Act.Relu, bias=bias_tiles[j][:], scale=float(sc)
                )
                relu_tiles.append(rt)

            # (sign, AP) left to fold into the accumulator with add/sub
            pending = [
                (sign, rt)
                for (sc, bi, sign), rt in zip(act_scales, relu_tiles)
            ]
            if base is not None:
                a, b = base
                bt = mpool.tile([P, F], f32, name="bt", tag="bt")
                nc.vector.tensor_scalar(
                    bt[:], xt[:], float(a), float(b), Alu.mult, Alu.add
                )
                pending.append((1, bt))

            if fold_jump:
                # last relu tile: t = (x >= Br) +/- relu_scaled, in place
                sgn_last, rt_last = pending[len(relu_tiles) - 1]
                pending.pop(len(relu_tiles) - 1)
                Brv, J = jump
                nc.vector.scalar_tensor_tensor(
                    rt_last[:], xt[:], float(Brv), rt_last[:],
                    Alu.is_ge, Alu.add if sgn_last > 0 else Alu.subtract,
                )
                if pending:
                    # acc = (t * J) + first_pending, into the pending tile if
                    # positive else into rt_last
                    sgn0, t0 = pending.pop(0)
                    if sgn0 > 0:
                        acc = t0
                        nc.vector.scalar_tensor_tensor(
                            acc[:], rt_last[:], float(J), t0[:], Alu.mult, Alu.add
                        )
                    else:
                        acc = rt_last
                        nc.vector.scalar_tensor_tensor(
                            acc[:], rt_last[:], float(J), t0[:], Alu.mult, Alu.subtract
                        )
                else:
                    acc = rt_last
                    nc.vector.tensor_scalar(
                        acc[:], rt_last[:], float(J), None, Alu.mult
                    )
            elif jump is not None:
                # no relu terms to fold into: plain masked jump
                acc = mpool.tile([P, F], f32, name="mt", tag="mt")
                Brv, J = jump
                nc.vector.tensor_scalar(
                    acc[:], xt[:], float(Brv), float(J), Alu.is_ge, Alu.mult
                )
            elif pending:
                sgn0, acc = pending.pop(0)
                if sgn0 < 0:
                    neg = acc
                    acc = mpool.tile([P, F], f32, name="nt", tag="nt")
                    nc.vector.tensor_scalar(acc[:], neg[:], -1.0, None, Alu.mult)
            else:
                acc = mpool.tile([P, F], f32, name="zt", tag="zt")
                nc.vector.memset(acc[:], 0.0)

            for sgn, t in pending:
                if sgn > 0:
                    nc.vector.tensor_add(acc[:], acc[:], t[:])
                else:
                    nc.vector.tensor_sub(acc[:], acc[:], t[:])

            nc.sync.dma_start(y_d[:, bass.ts(i, F)], acc[:])

    nc.compile()
    return nc


def _get_program(terms, base, jump, FT, repeat=1):
    key = (tuple(terms), base, jump, FT, repeat, F_OVERRIDE, BUFS, BUFS_X, BUFS_R)
    if key not in _PROGRAM_CACHE:
        _PROGRAM_CACHE[key] = _build_program(terms, base, jump, FT, repeat)
    return _PROGRAM_CACHE[key]


def kernel(x, N, Bounds, BoundSlope, nheight):
    global LAST_RESULTS
    from concourse.bass_utils import run_bass_kernel_spmd

    x = np.ascontiguousarray(np.asarray(x, dtype=np.float32))
    orig_shape = x.shape
    E = x.size
    pad = (-E) % (N_CORES * P)
    flat = x.reshape(-1)
    if pad:
        flat = np.concatenate([flat, np.zeros(pad, np.float32)])
    FT = (E + pad) // (N_CORES * P)

    terms, base, jump = _plan_params(
        np.asarray(N), np.asarray(Bounds), np.asarray(BoundSlope), np.asarray(nheight)
    )
    nc = _get_program(terms, base, jump, FT)

    shards = flat.reshape(N_CORES, P, FT)
    in_maps = [{"x": shards[i]} for i in range(N_CORES)]
    res = run_bass_kernel_spmd(
        nc, in_maps, core_ids=list(range(N_CORES)), trace=TRACE
    )
    LAST_RESULTS = res
    out = np.stack([r["y"] for r in res.results], axis=0).reshape(-1)
    if pad:
        out = out[:E]
    return out.reshape(orig_shape)


# revision 28
# speedup vs baseline: 1.1924x; 1.1924x over previous
"""Trainium2 Bass kernel for PiecewiseLinearUnitV2 (elementwise piecewise-linear unit).

Contract: kernel(**inputs) takes the FULL (unsharded) numpy inputs and returns
the FULL output. Internally the input batch is data-parallel sharded across 8
NeuronCores; the ~25-float parameter tensors are folded into compile-time
immediates on the host.

Math: the reference computes, per element x,
    y = b1*l1 + b2*l2 + b3*l3
with uniform bins between Bounds[0]=Bl and Bounds[1]=Br. That is a piecewise
linear function of x: continuous at Bl and at all interior knots, with a jump
J = nheight[I+1] - nheight[I] at Br. So it decomposes exactly as
    y = Kl*x + (nh0 - Kl*Bl)
        + sum_k d_k * relu(x - c_k)        (slope changes at Bl + k*IL)
        + (Kr - s_{I-1}) * relu(x - Br)
        + J * (x >= Br)
Terms with negligible |d_k| are dropped (for linspace nheight all interior
slope-changes vanish, leaving a 3-piece function). The relus run on ScalarE
(ACT) with the coefficient folded into the activation scale/bias. On VectorE
the jump mask and the sums are fused into two scalar_tensor_tensor ops:
    t = (x >= Br) + relu(|d/J|*x + b)      (is_ge, add)
    y = (t * J) + relu(d0*x + b0)          (mult, add)
Measured on HW: ~71-82us/core, at parity with a pure DMA in+out copy of the
same data (the HBM roofline, ~358 GB/s/core shared R+W).
"""

import numpy as np

P = 128
N_CORES = 8
MAX_N = 20

# Set by test harness to request an NTFF profile; results land in LAST_RESULTS.
TRACE = False
LAST_RESULTS = None

_PROGRAM_CACHE = {}


def _plan_params(N, Bounds, BoundSlope, nheight):
    """Mirror the reference's float32 arithmetic to derive the relu-sum
    coefficients. Returns (terms, base, jump) with plain-float entries:
      terms: [(d, c)]  ->  d * relu(x - c)
      base:  (a, b)    ->  a*x + b        (None if exactly zero)
      jump:  (Br, J)   ->  J * (x >= Br)  (None if J == 0)
    """
    f32 = np.float32
    intervals = f32(np.floor(np.clip(f32(N), f32(3.0), f32(MAX_N))))
    I = int(intervals)
    Bl, Br = f32(Bounds[0]), f32(Bounds[1])
    Kl, Kr = f32(BoundSlope[0]), f32(BoundSlope[1])
    nh = np.asarray(nheight, dtype=np.float32)
    IL = f32((Br - Bl) / intervals)

    s = [f32((nh[k + 1] - nh[k]) / IL) for k in range(I)]
    cs = [f32(f32(k) * IL + Bl) for k in range(I)] + [Br]
    ds = [f32(s[0] - Kl)] + [f32(s[k] - s[k - 1]) for k in range(1, I)]
    ds.append(f32(Kr - s[I - 1]))
    # jnp clamps out-of-bounds gathers, so nheight[I+1] at I==MAX_N reads nh[MAX_N]
    J = f32(nh[min(I + 1, MAX_N)] - nh[I])

    dmax = max([abs(float(d)) for d in ds] + [1e-30])
    terms = [
        (float(d), float(c))
        for d, c in zip(ds, cs)
        if abs(float(d)) > 1e-6 * max(dmax, 1.0)
    ]
    base_a = float(Kl)
    base_b = float(f32(nh[0] - f32(Kl * Bl)))
    base = None if (base_a == 0.0 and base_b == 0.0) else (base_a, base_b)
    jump = None if float(J) == 0.0 else (float(Br), float(J))
    return terms, base, jump


def _pick_tile_free_dim(FT, n_slots, budget_bytes=int(22.5 * 1024 * 1024)):
    """Largest even divisor of FT such that n_slots tiles of [128, F] f32 fit
    in the SBUF budget."""
    fmax = budget_bytes // (P * 4 * n_slots)
    best_even, best_any = 0, 0
    for f in range(1, FT + 1):
        if FT % f == 0 and f <= fmax and f <= 8192:
            best_any = max(best_any, f)
            if f % 2 == 0:  # even free dim enables DVE 2x modes
                best_even = max(best_even, f)
    best = best_even or best_any
    assert best > 0, f"no usable tile size for FT={FT}, slots={n_slots}"
    return best


# Tile sizing (bench.py overrides these for experiments). Measured on HW:
# F=6272 with 3 input bufs / 2 relu bufs runs at the HBM roofline (~71us/core);
# smaller tiles pay per-instruction gaps on DVE/ACT.
F_OVERRIDE = None
BUFS = 2
BUFS_X = 3
BUFS_R = None


def _build_program(terms, base, jump, FT, repeat=1):
    from contextlib import ExitStack

    import concourse.bass as bass
    import concourse.tile as tile
    from concourse import bacc
    import concourse.mybir as mybir

    Alu = mybir.AluOpType
    Act = mybir.ActivationFunctionType
    f32 = mybir.dt.float32
    f32np = np.float32

    bufs_x = BUFS_X or BUFS
    bufs_r = BUFS_R or BUFS
    # SBUF slot count: x + per-relu tiles + 2 for the misc pool
    n_slots = bufs_x + bufs_r * max(len(terms), 1) + 2 * (
        (jump is not None) + (base is not None)
    )
    F = F_OVERRIDE or _pick_tile_free_dim(FT, n_slots)
    n_tiles = FT // F

    nc = bacc.Bacc("TRN2", target_bir_lowering=False, debug=False, num_devices=N_CORES)
    x_d = nc.dram_tensor("x", [P, FT], f32, kind="ExternalInput").ap()
    y_d = nc.dram_tensor("y", [P, FT], f32, kind="ExternalOutput").ap()

    with tile.TileContext(nc) as tc, ExitStack() as ctx:
        xpool = ctx.enter_context(tc.tile_pool(name="xpool", bufs=bufs_x))
        rpool = ctx.enter_context(tc.tile_pool(name="rpool", bufs=bufs_r))
        mpool = ctx.enter_context(tc.tile_pool(name="mpool", bufs=2))
        cpool = ctx.enter_context(tc.tile_pool(name="cpool", bufs=1))

        # Decide the ACT scale for each relu term. If there is a jump, the
        # LAST term's relu is emitted pre-divided by J so the jump mask can be
        # fused in for free:
        #   t = (x >= Br) +/- relu(|d/J|*x + bias)      (one stt, is_ge+add)
        #   y = (t * J) + next                          (one stt, mult+add)
        # All other terms carry |d| inside the ACT scale and are combined with
        # add/sub tensor_tensor ops.
        fold_jump = (
            jump is not None
            and len(terms) > 0
            and 1e-4 <= abs(terms[-1][0] / jump[1]) <= 1e4
        )
        act_scales = []  # (scale, bias, sign_for_combine)
        for j, (d, c) in enumerate(terms):
            if fold_jump and j == len(terms) - 1:
                ratio = f32np(d) / f32np(jump[1])
                sc = abs(ratio)
                sign = 1 if ratio > 0 else -1
            else:
                sc = abs(f32np(d))
                sign = 1 if d > 0 else -1
            bi = -f32np(f32np(sc) * f32np(c))
            act_scales.append((float(sc), float(bi), sign))

        # per-term [P,1] bias tiles for the ACT relus (bias must be an AP)
        bias_tiles = []
        for j, (sc, bi, sign) in enumerate(act_scales):
            bias_t = cpool.tile([P, 1], f32, name=f"bias{j}", tag=f"bias{j}")
            nc.vector.memset(bias_t[:], float(bi))
            bias_tiles.append(bias_t)

        for i in [t for _ in range(repeat) for t in range(n_tiles)]:
            xt = xpool.tile([P, F], f32, name="xt", tag="xt")
            nc.sync.dma_start(xt[:], x_d[:, bass.ts(i, F)])

            relu_tiles = []
            for j, (sc, bi, sign) in enumerate(act_scales):
                rt = rpool.tile([P, F], f32, name=f"rt{j}", tag=f"rt{j}", bufs=bufs_r)
                nc.scalar.activation(
                    rt[:], xt[:], Act.Relu, bias=bias_tiles[j][:], scale=float(sc)
                )
                relu_tiles.append(rt)

            # (sign, AP) left to fold into the accumulator with add/sub
            pending = [
                (sign, rt)
                for (sc, bi, sign), rt in zip(act_scales, relu_tiles)
            ]
            if base is not None:
                a, b = base
                bt = mpool.tile([P, F], f32, name="bt", tag="bt")
                nc.vector.tensor_scalar(
                    bt[:], xt[:], float(a), float(b), Alu.mult, Alu.add
                )
                pending.append((1, bt))

            if fold_jump:
                # last relu tile: t = (x >= Br) +/- relu_scaled, in place
                sgn_last, rt_last = pending[len(relu_tiles) - 1]
                pending.pop(len(relu_tiles) - 1)
                Brv, J = jump
                nc.vector.scalar_tensor_tensor(
                    rt_last[:], xt[:], float(Brv), rt_last[:],
                    Alu.is_ge, Alu.add if sgn_last > 0 else Alu.subtract,
                )
                if pending:
                    # acc = (t * J) + first_pending, into the pending tile if
                    # positive else into rt_last
                    sgn0, t0 = pending.pop(0)
                    if sgn0 > 0:
                        acc = t0
                        nc.vector.scalar_tensor_tensor(
                            acc[:], rt_last[:], float(J), t0[:], Alu.mult, Alu.add
                        )
                    else:
                        acc = rt_last
                        nc.vector.scalar_tensor_tensor(
                            acc[:], rt_last[:], float(J), t0[:], Alu.mult, Alu.subtract
                        )
                else:
                    acc = rt_last
                    nc.vector.tensor_scalar(
                        acc[:], rt_last[:], float(J), None, Alu.mult
                    )
            elif jump is not None:
                # no relu terms to fold into: plain masked jump
                acc = mpool.tile([P, F], f32, name="mt", tag="mt")
                Brv, J = jump
                nc.vector.tensor_scalar(
                    acc[:], xt[:], float(Brv), float(J), Alu.is_ge, Alu.mult
                )
            elif pending:
                sgn0, acc = pending.pop(0)
                if sgn0 < 0:
                    neg = acc
                    acc = mpool.tile([P, F], f32, name="nt", tag="nt")
                    nc.vector.tensor_scalar(acc[:], neg[:], -1.0, None, Alu.mult)
            else:
                acc = mpool.tile([P, F], f32, name="zt", tag="zt")
                nc.vector.memset(acc[:], 0.0)

            for sgn, t in pending:
                if sgn > 0:
                    nc.vector.tensor_add(acc[:], acc[:], t[:])
                else:
                    nc.vector.tensor_sub(acc[:], acc[:], t[:])

            nc.sync.dma_start(y_d[:, bass.ts(i, F)], acc[:])

    nc.compile()
    return nc


def _get_program(terms, base, jump, FT, repeat=1):
    key = (tuple(terms), base, jump, FT, repeat, F_OVERRIDE, BUFS, BUFS_X, BUFS_R)
    if key not in _PROGRAM_CACHE:
        _PROGRAM_CACHE[key] = _build_program(terms, base, jump, FT, repeat)
    return _PROGRAM_CACHE[key]


def kernel(x, N, Bounds, BoundSlope, nheight):
    global LAST_RESULTS
    from concourse.bass_utils import run_bass_kernel_spmd

    x = np.ascontiguousarray(np.asarray(x, dtype=np.float32))
    orig_shape = x.shape
    E = x.size
    pad = (-E) % (N_CORES * P)
    flat = x.reshape(-1)
    if pad:
        flat = np.concatenate([flat, np.zeros(pad, np.float32)])
    FT = (E + pad) // (N_CORES * P)

    terms, base, jump = _plan_params(
        np.asarray(N), np.asarray(Bounds), np.asarray(BoundSlope), np.asarray(nheight)
    )
    nc = _get_program(terms, base, jump, FT)

    shards = flat.reshape(N_CORES, P, FT)
    in_maps = [{"x": shards[i]} for i in range(N_CORES)]
    res = run_bass_kernel_spmd(
        nc, in_maps, core_ids=list(range(N_CORES)), trace=TRACE
    )
    LAST_RESULTS = res
    out = np.stack([r["y"] for r in res.results], axis=0).reshape(-1)
    if pad:
        out = out[:E]
    return out.reshape(orig_shape)


# revision 32
# speedup vs baseline: 1.4753x; 1.2372x over previous
"""Trainium2 Bass kernel for PiecewiseLinearUnitV2 (elementwise piecewise-linear unit).

Contract: kernel(**inputs) takes the FULL (unsharded) numpy inputs and returns
the FULL output. Internally the input batch is data-parallel sharded across 8
NeuronCores; the ~25-float parameter tensors are folded into compile-time
immediates on the host.

Math: the reference computes, per element x,
    y = b1*l1 + b2*l2 + b3*l3
with uniform bins between Bounds[0]=Bl and Bounds[1]=Br. That is a piecewise
linear function of x: continuous at Bl and at all interior knots, with a jump
J = nheight[I+1] - nheight[I] at Br. So it decomposes exactly as
    y = Kl*x + (nh0 - Kl*Bl)
        + sum_k d_k * relu(x - c_k)        (slope changes at Bl + k*IL)
        + (Kr - s_{I-1}) * relu(x - Br)
        + J * (x >= Br)
Terms with negligible |d_k| are dropped (for linspace nheight all interior
slope-changes vanish, leaving a 3-piece function). The relus run on ScalarE
(ACT) with the coefficient folded into the activation scale/bias. On VectorE
the jump mask and the sums are fused into two scalar_tensor_tensor ops:
    t = (x >= Br) + relu(|d/J|*x + b)      (is_ge, add)
    y = (t * J) + relu(d0*x + b0)          (mult, add)
Measured on HW: ~71-82us/core, at parity with a pure DMA in+out copy of the
same data (the HBM roofline, ~358 GB/s/core shared R+W).
"""

import numpy as np

P = 128
N_CORES = 8
MAX_N = 20

# Set by test harness to request an NTFF profile; results land in LAST_RESULTS.
TRACE = False
LAST_RESULTS = None

_PROGRAM_CACHE = {}


def _plan_params(N, Bounds, BoundSlope, nheight):
    """Mirror the reference's float32 arithmetic to derive the relu-sum
    coefficients. Returns (terms, base, jump) with plain-float entries:
      terms: [(d, c)]  ->  d * relu(x - c)
      base:  (a, b)    ->  a*x + b        (None if exactly zero)
      jump:  (Br, J)   ->  J * (x >= Br)  (None if J == 0)
    """
    f32 = np.float32
    intervals = f32(np.floor(np.clip(f32(N), f32(3.0), f32(MAX_N))))
    I = int(intervals)
    Bl, Br = f32(Bounds[0]), f32(Bounds[1])
    Kl, Kr = f32(BoundSlope[0]), f32(BoundSlope[1])
    nh = np.asarray(nheight, dtype=np.float32)
    IL = f32((Br - Bl) / intervals)

    s = [f32((nh[k + 1] - nh[k]) / IL) for k in range(I)]
    cs = [f32(f32(k) * IL + Bl) for k in range(I)] + [Br]
    ds = [f32(s[0] - Kl)] + [f32(s[k] - s[k - 1]) for k in range(1, I)]
    ds.append(f32(Kr - s[I - 1]))
    # jnp clamps out-of-bounds gathers, so nheight[I+1] at I==MAX_N reads nh[MAX_N]
    J = f32(nh[min(I + 1, MAX_N)] - nh[I])

    dmax = max([abs(float(d)) for d in ds] + [1e-30])
    terms = [
        (float(d), float(c))
        for d, c in zip(ds, cs)
        if abs(float(d)) > 1e-6 * max(dmax, 1.0)
    ]
    base_a = float(Kl)
    base_b = float(f32(nh[0] - f32(Kl * Bl)))
    base = None if (base_a == 0.0 and base_b == 0.0) else (base_a, base_b)
    jump = None if float(J) == 0.0 else (float(Br), float(J))
    return terms, base, jump


def _pick_tile_free_dim(FT, n_slots, budget_bytes=int(22.5 * 1024 * 1024)):
    """Largest even divisor of FT such that n_slots tiles of [128, F] f32 fit
    in the SBUF budget."""
    fmax = budget_bytes // (P * 4 * n_slots)
    best_even, best_any = 0, 0
    for f in range(1, FT + 1):
        if FT % f == 0 and f <= fmax and f <= 8192:
            best_any = max(best_any, f)
            if f % 2 == 0:  # even free dim enables DVE 2x modes
                best_even = max(best_even, f)
    best = best_even or best_any
    assert best > 0, f"no usable tile size for FT={FT}, slots={n_slots}"
    return best


# Tile sizing (bench.py overrides these for experiments). Measured on HW:
# F=6272 with 3 input bufs / 2 relu bufs runs at the HBM roofline (~71us/core);
# smaller tiles pay per-instruction gaps on DVE/ACT.
F_OVERRIDE = None
BUFS = 2
BUFS_X = 3
BUFS_R = None
BUFS_R2 = None  # bufs for non-first relu tiles (staged mode squeezes these)
# STAGED: keep the whole per-core input resident in one SBUF buffer, compute
# in place, and DMA out from the same buffer. Separates the HBM channel into
# long read bursts and long write bursts (fewer R/W turnarounds) and removes
# buffer-recycling WAR stalls. Falls back to pipelined mode if it can't fit.
STAGED = False


def _build_program(terms, base, jump, FT, repeat=1):
    from contextlib import ExitStack

    import concourse.bass as bass
    import concourse.tile as tile
    from concourse import bacc
    import concourse.mybir as mybir

    Alu = mybir.AluOpType
    Act = mybir.ActivationFunctionType
    f32 = mybir.dt.float32
    f32np = np.float32

    bufs_x = BUFS_X or BUFS
    bufs_r = BUFS_R or BUFS
    bufs_r2 = BUFS_R2 or bufs_r
    budget = int(22.5 * 1024 * 1024)
    n_relu = max(len(terms), 1)
    staged = STAGED
    if staged:
        # whole input resident: working tiles must fit next to FT*512 bytes
        work_budget = budget - FT * P * 4
        n_slots = bufs_r + bufs_r2 * (n_relu - 1) + 2 * (base is not None) + 1
        staged = work_budget > 0 and work_budget // (P * 4 * n_slots) >= 512
        if staged:
            F = F_OVERRIDE or _pick_tile_free_dim(FT, n_slots, work_budget)
    if not staged:
        # pipelined: x tiles cycle through bufs_x slots
        n_slots = bufs_x + bufs_r + bufs_r2 * (n_relu - 1) + 2 * (
            (jump is not None) + (base is not None)
        )
        F = F_OVERRIDE or _pick_tile_free_dim(FT, n_slots, budget)
    n_tiles = FT // F

    nc = bacc.Bacc("TRN2", target_bir_lowering=False, debug=False, num_devices=N_CORES)
    x_d = nc.dram_tensor("x", [P, FT], f32, kind="ExternalInput").ap()
    y_d = nc.dram_tensor("y", [P, FT], f32, kind="ExternalOutput").ap()

    with tile.TileContext(nc) as tc, ExitStack() as ctx:
        xpool = ctx.enter_context(tc.tile_pool(name="xpool", bufs=1 if staged else bufs_x))
        rpool = ctx.enter_context(tc.tile_pool(name="rpool", bufs=bufs_r))
        mpool = ctx.enter_context(tc.tile_pool(name="mpool", bufs=2))
        cpool = ctx.enter_context(tc.tile_pool(name="cpool", bufs=1))

        xfull = None
        if staged:
            xfull = xpool.tile([P, FT], f32, name="xfull", tag="xfull")

        # Decide the ACT scale for each relu term. If there is a jump, the
        # LAST term's relu is emitted pre-divided by J so the jump mask can be
        # fused in for free:
        #   t = (x >= Br) +/- relu(|d/J|*x + bias)      (one stt, is_ge+add)
        #   y = (t * J) + next                          (one stt, mult+add)
        # All other terms carry |d| inside the ACT scale and are combined with
        # add/sub tensor_tensor ops.
        fold_jump = (
            jump is not None
            and len(terms) > 0
            and 1e-4 <= abs(terms[-1][0] / jump[1]) <= 1e4
        )
        act_scales = []  # (scale, bias, sign_for_combine)
        for j, (d, c) in enumerate(terms):
            if fold_jump and j == len(terms) - 1:
                ratio = f32np(d) / f32np(jump[1])
                sc = abs(ratio)
                sign = 1 if ratio > 0 else -1
            else:
                sc = abs(f32np(d))
                sign = 1 if d > 0 else -1
            bi = -f32np(f32np(sc) * f32np(c))
            act_scales.append((float(sc), float(bi), sign))

        # per-term [P,1] bias tiles for the ACT relus (bias must be an AP)
        bias_tiles = []
        for j, (sc, bi, sign) in enumerate(act_scales):
            bias_t = cpool.tile([P, 1], f32, name=f"bias{j}", tag=f"bias{j}")
            nc.vector.memset(bias_t[:], float(bi))
            bias_tiles.append(bias_t)

        for i in [t for _ in range(repeat) for t in range(n_tiles)]:
            if staged:
                xt = xfull[:, bass.ts(i % n_tiles, F)]
            else:
                xt = xpool.tile([P, F], f32, name="xt", tag="xt")
            nc.sync.dma_start(xt[:], x_d[:, bass.ts(i, F)])

            relu_tiles = []
            for j, (sc, bi, sign) in enumerate(act_scales):
                rt = rpool.tile(
                    [P, F], f32, name=f"rt{j}", tag=f"rt{j}",
                    bufs=bufs_r if j == 0 else bufs_r2,
                )
                nc.scalar.activation(
                    rt[:], xt[:], Act.Relu, bias=bias_tiles[j][:], scale=float(sc)
                )
                relu_tiles.append(rt)

            # (sign, AP) left to fold into the accumulator with add/sub
            pending = [
                (sign, rt)
                for (sc, bi, sign), rt in zip(act_scales, relu_tiles)
            ]
            if base is not None:
                a, b = base
                bt = mpool.tile([P, F], f32, name="bt", tag="bt")
                nc.vector.tensor_scalar(
                    bt[:], xt[:], float(a), float(b), Alu.mult, Alu.add
                )
                pending.append((1, bt))

            # `target` is where the final value accumulates (and what DMAs
            # out). Staged mode reuses the x slice — x is dead once the relus
            # and the is_ge mask have read it, and Tile orders that via WAR.
            target = xt if staged else None

            if fold_jump:
                # last relu tile: t = (x >= Br) +/- relu_scaled, in place
                sgn_last, rt_last = pending.pop(len(relu_tiles) - 1)
                Brv, J = jump
                nc.vector.scalar_tensor_tensor(
                    rt_last[:], xt[:], float(Brv), rt_last[:],
                    Alu.is_ge, Alu.add if sgn_last > 0 else Alu.subtract,
                )
                if target is None:
                    target = rt_last
                if pending:
                    sgn0, t0 = pending.pop(0)
                    nc.vector.scalar_tensor_tensor(
                        target[:], rt_last[:], float(J), t0[:],
                        Alu.mult, Alu.add if sgn0 > 0 else Alu.subtract,
                    )
                else:
                    nc.vector.tensor_scalar(
                        target[:], rt_last[:], float(J), None, Alu.mult
                    )
            elif jump is not None:
                # no relu terms to fold into: plain masked jump
                Brv, J = jump
                if target is None:
                    target = mpool.tile([P, F], f32, name="mt", tag="mt")
                nc.vector.tensor_scalar(
                    target[:], xt[:], float(Brv), float(J), Alu.is_ge, Alu.mult
                )
            elif pending:
                sgn0, t0 = pending.pop(0)
                if pending:
                    sgn1, t1 = pending.pop(0)
                    if target is None:
                        target = t0
                    if sgn0 > 0 and sgn1 > 0:
                        nc.vector.tensor_add(target[:], t0[:], t1[:])
                    elif sgn0 > 0:
                        nc.vector.tensor_sub(target[:], t0[:], t1[:])
                    elif sgn1 > 0:
                        nc.vector.tensor_sub(target[:], t1[:], t0[:])
                    else:
                        nc.vector.tensor_scalar(target[:], t0[:], -1.0, None, Alu.mult)
                        nc.vector.tensor_sub(target[:], target[:], t1[:])
                elif target is None and sgn0 > 0:
                    target = t0
                else:
                    if target is None:
                        target = mpool.tile([P, F], f32, name="nt", tag="nt")
                    nc.vector.tensor_scalar(
                        target[:], t0[:], 1.0 if sgn0 > 0 else -1.0, None, Alu.mult
                    )
            else:
                if target is None:
                    target = mpool.tile([P, F], f32, name="zt", tag="zt")
                nc.vector.memset(target[:], 0.0)

            for sgn, t in pending:
                if sgn > 0:
                    nc.vector.tensor_add(target[:], target[:], t[:])
                else:
                    nc.vector.tensor_sub(target[:], target[:], t[:])

            nc.sync.dma_start(y_d[:, bass.ts(i, F)], target[:])

    nc.compile()
    return nc


def _get_program(terms, base, jump, FT, repeat=1):
    key = (
        tuple(terms), base, jump, FT, repeat,
        F_OVERRIDE, BUFS, BUFS_X, BUFS_R, BUFS_R2, STAGED,
    )
    if key not in _PROGRAM_CACHE:
        _PROGRAM_CACHE[key] = _build_program(terms, base, jump, FT, repeat)
    return _PROGRAM_CACHE[key]


def kernel(x, N, Bounds, BoundSlope, nheight):
    global LAST_RESULTS
    from concourse.bass_utils import run_bass_kernel_spmd

    x = np.ascontiguousarray(np.asarray(x, dtype=np.float32))
    orig_shape = x.shape
    E = x.size
    pad = (-E) % (N_CORES * P)
    flat = x.reshape(-1)
    if pad:
        flat = np.concatenate([flat, np.zeros(pad, np.float32)])
    FT = (E + pad) // (N_CORES * P)

    terms, base, jump = _plan_params(
        np.asarray(N), np.asarray(Bounds), np.asarray(BoundSlope), np.asarray(nheight)
    )
    nc = _get_program(terms, base, jump, FT)

    shards = flat.reshape(N_CORES, P, FT)
    in_maps = [{"x": shards[i]} for i in range(N_CORES)]
    res = run_bass_kernel_spmd(
        nc, in_maps, core_ids=list(range(N_CORES)), trace=TRACE
    )
    LAST_RESULTS = res
    out = np.stack([r["y"] for r in res.results], axis=0).reshape(-1)
    if pad:
        out = out[:E]
    return out.reshape(orig_shape)
